# revision 5
# baseline (speedup 1.0000x reference)
"""Single-head causal attention (B=8, T=2048, D=512, H=64) on 8 TRN2 cores.

Data-parallel: one batch element per NeuronCore. Each core computes
attention in the S^T layout (keys on partitions, queries on the free axis):

  qT/kT/vT [64, T] = W.T @ x.T        (fp16 matmuls, 4 c-tile accumulation)
  v1       [128, 16, 65]              (DMA-engine transpose of vT + ones col)
  S^T[j,i] = kT_jblock.T @ qT          (strips of causal width)
  P^T      = exp(S^T / 8)              (ScalarE, one op per strip;
                                        no max-subtraction: scores are
                                        bounded by ~|q||k|sqrt(H)/8 << 88)
  out^T[h,i], l[i] = [v|1]_jb.T @ P^T  (accumulated over j-blocks in PSUM;
                                        row 64 is the softmax denominator)

The kernel DMAs the unnormalized [65, T] strip straight from PSUM; the
host divides by the denominator row and transposes back to [T, 64].

v2 notes vs v1: chunk-major x layout (one contiguous DMA piece per
512-col chunk, split across the qSP/qAct HWDGE queues), PE p-state held
up by a dense warm-up burst, V transposed on the DMA xbar instead of the
PE, outputs DMA'd directly from PSUM, and a software pipeline that keeps
the ACT (exp) engine saturated through both attention passes.
"""

import sys

sys.path.insert(0, "/opt/trn_rl_repo")

import numpy as np

import concourse.bass as bass
import concourse.mybir as mybir
import concourse.tile as tile

B, T, D, H = 8, 2048, 512, 64
N_CORES = 8
HALF = T // 2  # i-axis pass width
NC_TILES = D // 128  # 4 c-tiles
NCH = T // 512  # 4 t-chunks

f32 = mybir.dt.float32
f16 = mybir.dt.float16
bf16 = mybir.dt.bfloat16

_cache = {}


def _legalize_waits(nc, max_waits=1):
    """Walrus codegen accepts at most one sync wait per instruction; hoist
    extras onto same-engine NOPs placed immediately before (engine queues
    are FIFO so blocking semantics are unchanged)."""
    counter = 0
    for bb in nc.main_func.blocks:
        if not any(
            ins.sync_info is not None and len(ins.sync_info.on_wait) > max_waits
            for ins in bb.instructions
        ):
            continue
        new_list = []
        for ins in bb.instructions:
            si = ins.sync_info
            if si is not None and len(si.on_wait) > max_waits:
                waits = list(si.on_wait)
                hoist, keep = waits[:-max_waits], waits[-max_waits:]
                for w in hoist:
                    counter += 1
                    new_list.append(
                        mybir.InstNoOp(
                            name=f"I-waitfix-{counter}",
                            engine=ins.engine,
                            sync_info=mybir.SyncInfo(on_wait=[w], on_update=[]),
                            bass_nofuse=True,
                        )
                    )
                ins.sync_info = mybir.SyncInfo(
                    on_wait=keep, on_update=list(si.on_update)
                )
            new_list.append(ins)
        bb.instructions = new_list
    return counter


def _chunks(lo, hi, step, align):
    """Split [lo, hi) at multiples of `step` relative to `align`."""
    out = []
    cur = lo
    while cur < hi:
        nxt = min(hi, align + ((cur - align) // step + 1) * step)
        out.append((cur, nxt))
        cur = nxt
    return out


CW = 512 + 256 + 128 + 16  # wqk | wv | mask | ones


def _build():
    nc = bass.Bass()

    xhi_d = nc.declare_dram_parameter("xhi", [128, NCH * NC_TILES * 512], f16,
                                      isOutput=False)
    consts_d = nc.declare_dram_parameter("consts", [128, CW], f16, isOutput=False)
    out_d = nc.declare_dram_parameter("out", [H + 1, T], f32, isOutput=True)

    with tile.TileContext(nc) as tc:
        with (
            tc.tile_pool(name="const", bufs=1) as cpool,
            tc.tile_pool(name="xt", bufs=1) as xpool,
            tc.tile_pool(name="qkv", bufs=1) as qkvpool,
            tc.tile_pool(name="p", bufs=2) as ppool,
            tc.tile_pool(name="o", bufs=2) as opool,
            tc.tile_pool(name="ps_proj", bufs=2, space="PSUM") as ps_proj,
            tc.tile_pool(name="ps_s", bufs=2, space="PSUM") as ps_s,
            tc.tile_pool(name="ps_pv", bufs=1, space="PSUM") as ps_pv,
        ):
            # ---- constants / warm-up ----
            consts = cpool.tile([128, CW], f16)
            nc.scalar.dma_start(consts[:], consts_d[:])
            wqk = [consts[:, 128 * c : 128 * (c + 1)] for c in range(NC_TILES)]
            wv = [consts[:, 512 + 64 * c : 512 + 64 * (c + 1)] for c in range(NC_TILES)]
            mask16 = consts[:, 768:896]

            warm_bf = cpool.tile([128, 512], bf16)
            nc.gpsimd.memset(warm_bf[:], 1.0)
            # touch Exp so the ACT table loads during the DMA phase
            exp_warm = cpool.tile([1, 2], f32)
            nc.scalar.activation(
                exp_warm[:], warm_bf[0:1, 0:2], mybir.ActivationFunctionType.Exp
            )

            # ---- input DMAs: one piece per 512-col chunk, two queues ----
            xhi = xpool.tile([128, NCH, NC_TILES, 512], f16)
            xsrc = xhi_d.rearrange("p (k c t) -> p k c t", k=NCH, c=NC_TILES)
            for k in range(NCH):
                eng = nc.sync if k % 2 == 0 else nc.scalar
                eng.dma_start(xhi[:, k, :, :], xsrc[:, k, :, :])

            qT = qkvpool.tile([H, T], f16)
            kT = qkvpool.tile([H, T], f16)
            vT = qkvpool.tile([H, T], f16)
            v1 = qkvpool.tile([128, T // 128, H + 1], f16)
            nc.gpsimd.memset(v1[:, :, H : H + 1], 1.0)

            # ---- PE warm-up: hold the p-state up until x lands ----
            warm_ps = ps_s.tile([128, 512], f32, tag="s", name="warm_ps")
            for _ in range(8):
                nc.tensor.matmul(
                    warm_ps[:], warm_bf[:, 0:128], warm_bf[:], start=True, stop=True
                )

            # ---- projections ----
            def proj_qk(k):
                ps = ps_proj.tile([128, 512], f32, tag="work", name="qk_ps")
                for c in range(NC_TILES):
                    nc.tensor.matmul(
                        ps[:],
                        wqk[c],
                        xhi[:, k, c, :],
                        start=(c == 0),
                        stop=(c == NC_TILES - 1),
                    )
                nc.vector.tensor_copy(qT[:, 512 * k : 512 * (k + 1)], ps[0:H, :])
                nc.vector.tensor_copy(kT[:, 512 * k : 512 * (k + 1)], ps[H : 2 * H, :])

            def proj_v(k):
                ps = ps_proj.tile([128, 512], f32, tag="work", name="v_ps")
                for c in range(NC_TILES):
                    nc.tensor.matmul(
                        ps[0:H, :],
                        wv[c],
                        xhi[:, k, c, :],
                        start=(c == 0),
                        stop=(c == NC_TILES - 1),
                    )
                nc.vector.tensor_copy(vT[:, 512 * k : 512 * (k + 1)], ps[0:H, :])
                # transpose this 512-col chunk on the DMA xbar into a
                # contiguous staging tile (the xbar can't write the 65-wide
                # v1 rows), then strided-copy into v1 on the idle Pool engine
                v1s = opool.tile([128, 4, H], f16, tag="v1s", name="v1s", bufs=2)
                nc.sync.dma_start_transpose(v1s[:], vT[:, 512 * k : 512 * (k + 1)])
                nc.gpsimd.tensor_copy(v1[:, 4 * k : 4 * (k + 1), 0:H], v1s[:])

            # ---- attention units ----
            def attn_S(t0, jb):
                i_start = max(t0, 128 * jb)
                W = t0 + HALF - i_start
                s_ps = ps_s.tile([128, 1024], f32, tag="s", name="s_ps")
                for ls, le in _chunks(0, W, 512, 0):
                    nc.tensor.matmul(
                        s_ps[:, ls:le],
                        kT[:, 128 * jb : 128 * (jb + 1)],
                        qT[:, i_start + ls : i_start + le],
                        start=True,
                        stop=True,
                    )
                return s_ps

            def attn_exp(t0, jb, s_ps):
                i_start = max(t0, 128 * jb)
                W = t0 + HALF - i_start
                p_sb = ppool.tile([128, 1024], f16, tag="p", name="p_sb", bufs=5)
                nc.scalar.activation(
                    p_sb[:, 0:W],
                    s_ps[:, 0:W],
                    mybir.ActivationFunctionType.Exp,
                    scale=1.0 / 8.0,
                )
                if 128 * jb >= t0:
                    nc.vector.tensor_mul(p_sb[:, 0:128], p_sb[:, 0:128], mask16)
                return p_sb

            def attn_pv(t0, n_jb, pv_ps, jb, p_sb):
                i_start = max(t0, 128 * jb)
                for gs, ge in _chunks(i_start, t0 + HALF, 512, 0):
                    ic_last_jb = min(n_jb - 1, (ge - 1) // 128)
                    nc.tensor.matmul(
                        pv_ps[:, gs - t0 : ge - t0],
                        v1[:, jb, :],
                        p_sb[:, gs - i_start : ge - i_start],
                        start=(jb == 0),
                        stop=(jb == ic_last_jb),
                    )

            def out_chunk(pv_ps, t0, c):
                out_sb = opool.tile([H + 1, 512], f32, tag="o", name="out_sb")
                nc.vector.tensor_copy(out_sb[:], pv_ps[:, 512 * c : 512 * (c + 1)])
                nc.sync.dma_start(
                    out_d[:, t0 + 512 * c : t0 + 512 * (c + 1)], out_sb[:]
                )

            # ---- phase 0: projections for chunks 0,1 ----
            proj_qk(0)
            proj_v(0)
            proj_qk(1)
            proj_v(1)

            # ---- attention pass 0 (i in [0, 1024)), j-blocks 0..7 ----
            # chunk 2/3 projection units woven into the PE stream
            weave0 = [
                lambda: proj_qk(2),
                lambda: proj_v(2),
                lambda: proj_qk(3),
                lambda: proj_v(3),
            ]
            pv_ps0 = ps_pv.tile([H + 1, HALF], f32, tag="pv", name="pv_ps")
            s_cur = attn_S(0, 0)
            for jb in range(8):
                s_nxt = attn_S(0, jb + 1) if jb + 1 < 8 else None
                p_sb = attn_exp(0, jb, s_cur)
                attn_pv(0, 8, pv_ps0, jb, p_sb)
                s_cur = s_nxt
                if weave0:
                    weave0.pop(0)()
                if jb == 3:
                    out_chunk(pv_ps0, 0, 0)
            out_chunk(pv_ps0, 0, 1)

            # ---- attention pass 1 (i in [1024, 2048)), j-blocks 0..15 ----
            pv_ps1 = ps_pv.tile([H + 1, HALF], f32, tag="pv", name="pv_ps")
            s_cur = attn_S(HALF, 0)
            for jb in range(16):
                s_nxt = attn_S(HALF, jb + 1) if jb + 1 < 16 else None
                p_sb = attn_exp(HALF, jb, s_cur)
                attn_pv(HALF, 16, pv_ps1, jb, p_sb)
                s_cur = s_nxt
                if jb == 11:
                    out_chunk(pv_ps1, HALF, 0)
            out_chunk(pv_ps1, HALF, 1)

    _legalize_waits(nc)
    return nc


def build_in_maps(x, Wq, Wk, Wv):
    x = np.ascontiguousarray(np.asarray(x), dtype=np.float32)
    wqk_np = np.ascontiguousarray(
        np.concatenate([np.asarray(Wq), np.asarray(Wk)], axis=1), dtype=np.float32
    )
    wv_np = np.ascontiguousarray(np.asarray(Wv), dtype=np.float32)

    def ctile_pack(a, w):  # [512, w] -> [128, 4*w] with c-tiles side by side
        return a.reshape(4, 128, w).transpose(1, 0, 2).reshape(128, 4 * w)

    mask_np = np.triu(np.ones((128, 128), dtype=np.float16))
    ones_np = np.ones((128, 16), dtype=np.float16)
    consts_np = np.ascontiguousarray(
        np.concatenate(
            [
                ctile_pack(wqk_np.astype(np.float16), 128),
                ctile_pack(wv_np.astype(np.float16), 64),
                mask_np,
                ones_np,
            ],
            axis=1,
        )
    )

    maps = []
    for b in range(N_CORES):
        # xhi[p, k, c, t] = x[b, 512k + t, 128c + p]
        xb = x[b].astype(np.float16)  # [T, D]
        xhi = np.ascontiguousarray(
            xb.reshape(NCH, 512, NC_TILES, 128).transpose(3, 0, 2, 1).reshape(128, -1)
        )
        maps.append({"xhi": xhi, "consts": consts_np})
    return maps


def kernel(x, Wq, Wk, Wv):
    from concourse.bass_utils import run_bass_kernel_spmd

    if "nc" not in _cache:
        _cache["nc"] = _build()
    nc = _cache["nc"]

    in_maps = build_in_maps(x, Wq, Wk, Wv)
    res = run_bass_kernel_spmd(nc, in_maps, list(range(N_CORES))).results

    out = np.empty((B, T, H), dtype=np.float32)
    for b in range(N_CORES):
        strip = res[b]["out"]  # [H+1, T]
        out[b] = (strip[:H, :] / strip[H : H + 1, :]).T
    return out


if __name__ == "__main__":
    rng = np.random.default_rng(0)
    x = rng.standard_normal((B, T, D)).astype(np.float32)
    s = 1.0 / np.sqrt(D)
    Wq = (rng.standard_normal((D, H)) * s).astype(np.float32)
    Wk = (rng.standard_normal((D, H)) * s).astype(np.float32)
    Wv = (rng.standard_normal((D, H)) * s).astype(np.float32)
    out = kernel(x=x, Wq=Wq, Wk=Wk, Wv=Wv)
    print("out", out.shape, out.dtype, np.abs(out).max())
